# revision 1
# baseline (speedup 1.0000x reference)
"""Detection-loss Bass kernel builder (shared by dev test and final kernel.py).

Layout: per image, anchors n = p*512 + f  (p in [0,128), f in [0,512)).
Each core processes IMGS=2 images; output = sum over its images of
(cls_loss + reg_loss).  Host divides the 8 partial sums by B=16.

Container-specific constraints honored here:
 - walrus supports only ONE sync-wait per instruction -> kernel.py splits
   multi-wait instructions into NoOp chains at BIR-JSON level.
 - extended q7 ops (partition_broadcast/all_reduce, local_scatter) are NOT
   supported -> use DMA broadcast (step-0 partition APs), PE-matmul column
   sums, and indirect-DMA gathers instead.
"""
import numpy as np
import concourse.bass as bass
import concourse.mybir as mybir
import concourse.bass_isa as bass_isa

F32 = mybir.dt.float32
I16 = mybir.dt.int16
I32 = mybir.dt.int32
U32 = mybir.dt.uint32
Alu = mybir.AluOpType
Act = mybir.ActivationFunctionType

P = 128
F = 512
N = P * F          # 65536
C = 21
M = 20
CC = 16            # pos candidates per partition
NW = 8             # neg candidates per partition
IMGS = 2           # images per core
PK = 6             # packed payload slots per anchor

POS_T, NEG_T = 0.5 / 1.5, 0.4 / 1.4   # r-space: r = inter/(An+Am), iou = r/(1-r)
ALPHA = 0.25
MIN_POS = 10.0
RATIO = 3.0
MR_IMM = -1.0e30   # match_replace fill


def build(nc, tc, outs, ins):
    v = nc.vector
    g = nc.gpsimd
    s = nc.scalar
    (o_part,) = outs
    d_cls, d_reg, d_anch, d_tb, d_tl, d_sc = ins

    with tc.tile_pool(name="main", bufs=1) as pl, \
         tc.tile_pool(name="io", bufs=1) as pio, \
         tc.tile_pool(name="lp", bufs=2) as plp, \
         tc.tile_pool(name="ps", bufs=2, space="PSUM") as pps, \
         tc.tile_pool(name="dr", bufs=1, space="DRAM") as pdr:

        # ---------- one-time setup ----------
        anch = pl.tile([P, 2048], F32, tag="anch")
        nc.sync.dma_start(anch[:], d_anch.rearrange("(p f) -> p f", p=P))
        a0 = anch[:, 0:2048:4]
        a1 = anch[:, 1:2048:4]
        a2 = anch[:, 2:2048:4]
        a3 = anch[:, 3:2048:4]
        aw = pl.tile([P, F], F32, tag="aw")
        ah = pl.tile([P, F], F32, tag="ah")
        ax = pl.tile([P, F], F32, tag="ax")
        ay = pl.tile([P, F], F32, tag="ay")
        v.tensor_tensor(aw[:], a2, a0, op=Alu.subtract)
        v.tensor_tensor(ah[:], a3, a1, op=Alu.subtract)
        v.scalar_tensor_tensor(ax[:], aw[:], 0.5, a0, op0=Alu.mult, op1=Alu.add)
        v.scalar_tensor_tensor(ay[:], ah[:], 0.5, a1, op0=Alu.mult, op1=Alu.add)

        ones = pl.tile([P, 1], F32, tag="ones")
        v.memset(ones[:], 1.0)
        iota_m_i = pl.tile([P, M], I32, tag="iomi")
        g.iota(iota_m_i[:], pattern=[[1, M]], base=0, channel_multiplier=0)
        iota_m = pl.tile([P, M], F32, tag="iom")        # 0..19 f32
        v.tensor_copy(iota_m[:], iota_m_i[:])
        iota_r_i = pl.tile([P, P], I32, tag="iori")
        g.iota(iota_r_i[:], pattern=[[1, P]], base=0, channel_multiplier=0)
        iota_r = pl.tile([P, P], F32, tag="ior")        # 0..127 f32
        v.tensor_copy(iota_r[:], iota_r_i[:])
        iota_f_i = pl.tile([P, F], I32, tag="iofi")
        g.iota(iota_f_i[:], pattern=[[1, F]], base=0, channel_multiplier=0)
        iota_f = pl.tile([P, F], F32, tag="iof")        # 0..511 f32
        v.tensor_copy(iota_f[:], iota_f_i[:])

        acc_part = pl.tile([P, 1], F32, tag="accp")     # per-core result accum
        v.memset(acc_part[:], 0.0)

        def psum_bcast(dst, src_cols, n):
            """dst[128, n] = column sums of src_cols[128, n], replicated."""
            pst = pps.tile([1, 8], F32, tag="pst")
            nc.tensor.matmul(pst[:, 0:n], ones[:], src_cols)
            row = pio.tile([1, 8], F32, tag="psrow")
            v.tensor_copy(row[:, 0:n], pst[:, 0:n])
            drow = pdr.tile([1, 8], F32, tag="psdr")
            nc.sync.dma_start(drow[:, 0:n], row[:, 0:n])
            nc.sync.dma_start(dst, drow[:, 0:n].broadcast_to([P, n]))

        for img in range(IMGS):
            # ---------- loads ----------
            regs = pio.tile([P, 4, F], F32, tag="regs")
            nc.sync.dma_start(
                regs[:], d_reg[img, :, :].rearrange("r (p f) -> p r f", p=P))
            cls = pio.tile([P, C, F], F32, tag="cls")
            nc.sync.dma_start(
                cls[:], d_cls[img, :, :].rearrange("c (p f) -> p c f", p=P))
            sc = pio.tile([P, F], F32, tag="sc")
            nc.sync.dma_start(sc[:], d_sc[img, :].rearrange("(p f) -> p f", p=P))
            bgt = pio.tile([P, 80], F32, tag="bgt")     # gt boxes replicated
            nc.sync.dma_start(
                bgt[:],
                d_tb[img, :, :].rearrange("m c -> (m c)")[None, :].broadcast_to([P, 80]))
            tli = pio.tile([1, M], I32, tag="tli")
            nc.sync.dma_start(tli[:], d_tl[img, :][None, :])
            tlf0 = pio.tile([1, M], F32, tag="tlf0")
            v.tensor_copy(tlf0[:], tli[:])
            dtl = pdr.tile([1, M], F32, tag="dtl")
            nc.sync.dma_start(dtl[:], tlf0[:])
            tlf = pio.tile([P, M], F32, tag="tlf")      # labels replicated f32
            nc.sync.dma_start(tlf[:], dtl[:].broadcast_to([P, M]))

            bx0 = bgt[:, 0:80:4]
            by0 = bgt[:, 1:80:4]
            bx1 = bgt[:, 2:80:4]
            by1 = bgt[:, 3:80:4]
            bw = pio.tile([P, M], F32, tag="bw")
            bh = pio.tile([P, M], F32, tag="bh")
            bA = pio.tile([P, M], F32, tag="bA")
            v.tensor_tensor(bw[:], bx1, bx0, op=Alu.subtract)
            v.tensor_tensor(bh[:], by1, by0, op=Alu.subtract)
            v.tensor_tensor(bA[:], bw[:], bh[:], op=Alu.mult)

            # ---------- decode ----------
            dx0t = pio.tile([P, F], F32, tag="dx0t")
            dx1t = pio.tile([P, F], F32, tag="dx1t")
            dy0t = pio.tile([P, F], F32, tag="dy0t")
            dy1t = pio.tile([P, F], F32, tag="dy1t")
            Ant = pio.tile([P, F], F32, tag="Ant")
            lset = pio.tile([P, F], F32, tag="lset")
            dx0 = dx0t[:]
            dx1 = dx1t[:]
            dy0 = dy0t[:]
            dy1 = dy1t[:]
            An = Ant[:]
            lse = lset[:]
            ew = pio.tile([P, F], F32, tag="ew")
            eh = pio.tile([P, F], F32, tag="eh")
            s.activation(ew[:], regs[:, 2, :], Act.Exp)
            s.activation(eh[:], regs[:, 3, :], Act.Exp)
            w = pio.tile([P, F], F32, tag="w")
            h = pio.tile([P, F], F32, tag="h")
            v.tensor_tensor(w[:], aw[:], ew[:], op=Alu.mult)
            v.tensor_tensor(h[:], ah[:], eh[:], op=Alu.mult)
            cx = pio.tile([P, F], F32, tag="cx")
            cy = pio.tile([P, F], F32, tag="cy")
            v.tensor_tensor(cx[:], regs[:, 0, :], aw[:], op=Alu.mult)
            v.tensor_tensor(cy[:], regs[:, 1, :], ah[:], op=Alu.mult)
            v.tensor_tensor(cx[:], cx[:], ax[:], op=Alu.add)
            v.tensor_tensor(cy[:], cy[:], ay[:], op=Alu.add)
            v.scalar_tensor_tensor(dx0, w[:], -0.5, cx[:], op0=Alu.mult, op1=Alu.add)
            v.scalar_tensor_tensor(dx1, w[:], 0.5, cx[:], op0=Alu.mult, op1=Alu.add)
            v.scalar_tensor_tensor(dy0, h[:], -0.5, cy[:], op0=Alu.mult, op1=Alu.add)
            v.scalar_tensor_tensor(dy1, h[:], 0.5, cy[:], op0=Alu.mult, op1=Alu.add)
            v.tensor_tensor(An, w[:], h[:], op=Alu.mult)

            # ---------- dense IoU max over M ----------
            mx = pio.tile([P, F], F32, tag="mx")
            v.memset(mx[:], 0.0)
            un = pio.tile([P, F], F32, tag="un")
            ur = pio.tile([P, F], F32, tag="ur")
            iwc = pio.tile([P, F], F32, tag="iwc")
            ihc = pio.tile([P, F], F32, tag="ihc")
            for m in range(M):
                ix0 = plp.tile([P, F], F32, tag="ix0")
                ix1 = plp.tile([P, F], F32, tag="ix1")
                iy0 = plp.tile([P, F], F32, tag="iy0")
                iy1 = plp.tile([P, F], F32, tag="iy1")
                rm = plp.tile([P, F], F32, tag="rm")
                iw = plp.tile([P, F], F32, tag="iw")
                ih = plp.tile([P, F], F32, tag="ih")
                inter = plp.tile([P, F], F32, tag="inter")
                v.tensor_scalar(ix0[:], dx0, bx0[:, m:m + 1], None, op0=Alu.max)
                v.tensor_scalar(ix1[:], dx1, bx1[:, m:m + 1], None, op0=Alu.min)
                g.tensor_tensor(iw[:], ix1[:], ix0[:], op=Alu.subtract)
                v.tensor_scalar(iy0[:], dy0, by0[:, m:m + 1], None, op0=Alu.max)
                v.tensor_scalar(iy1[:], dy1, by1[:, m:m + 1], None, op0=Alu.min)
                g.tensor_tensor(ih[:], iy1[:], iy0[:], op=Alu.subtract)
                s.activation(iwc[:], iw[:], Act.Relu)
                s.activation(ihc[:], ih[:], Act.Relu)
                v.tensor_tensor(inter[:], iwc[:], ihc[:], op=Alu.mult)
                v.tensor_scalar(un[:], An, bA[:, m:m + 1], None, op0=Alu.add)
                v.reciprocal(ur[:], un[:])
                g.tensor_tensor(rm[:], inter[:], ur[:], op=Alu.mult)
                v.tensor_tensor(mx[:], mx[:], rm[:], op=Alu.max)

            # ---------- LSE (no max-subtraction needed; |cls| < 6) ----------
            esum = pio.tile([P, F], F32, tag="esum")
            nchnk = 7
            for ci in range(nchnk):
                c0 = 3 * ci
                echnk = plp.tile([P, 3, F], F32, tag="echnk")
                s.activation(echnk[:], cls[:, c0:c0 + 3, :], Act.Exp)
                esrc = echnk[:].rearrange("p c f -> p f c")
                if ci == 0:
                    v.tensor_reduce(esum[:], esrc, axis=mybir.AxisListType.X, op=Alu.add)
                else:
                    epart = pio.tile([P, F], F32, tag="epart")
                    v.tensor_reduce(epart[:], esrc, axis=mybir.AxisListType.X, op=Alu.add)
                    g.tensor_tensor(esum[:], esum[:], epart[:], op=Alu.add)
            s.activation(lse, esum[:], Act.Ln)

            # ---------- counts ----------
            posr = pio.tile([P, F], F32, tag="junkF")
            negm = pio.tile([P, F], F32, tag="negm")
            cnt2 = pio.tile([P, 2], F32, tag="cnt2")
            v.tensor_scalar(posr[:], mx[:], POS_T, None, op0=Alu.is_ge,
                            op1=Alu.add, accum_out=cnt2[:, 0:1])
            v.tensor_scalar(negm[:], mx[:], NEG_T, None, op0=Alu.is_lt,
                            op1=Alu.add, accum_out=cnt2[:, 1:2])
            cnt2r = pio.tile([P, 2], F32, tag="cnt2r")
            psum_bcast(cnt2r[:], cnt2[:], 2)
            npos_raw = cnt2r[:, 0:1]
            nneg = cnt2r[:, 1:2]
            use_fb = pio.tile([P, 1], F32, tag="usefb")
            v.tensor_scalar(use_fb[:], npos_raw, MIN_POS, None, op0=Alu.is_lt)
            num_pos = pio.tile([P, 1], F32, tag="numpos")
            t1 = pio.tile([P, 1], F32, tag="t1")
            v.tensor_scalar(t1[:], npos_raw, -1.0, MIN_POS, op0=Alu.mult, op1=Alu.add)
            v.tensor_tensor(t1[:], t1[:], use_fb[:], op=Alu.mult)
            v.tensor_tensor(num_pos[:], npos_raw, t1[:], op=Alu.add)
            kk = pio.tile([P, 1], F32, tag="kk")
            v.tensor_scalar(kk[:], num_pos[:], RATIO, None, op0=Alu.mult)

            # ---------- pos candidates: top-16 mx per partition ----------
            mxc = pio.tile([P, F], F32, tag="mxc")
            v.tensor_copy(mxc[:], mx[:])
            V = pio.tile([P, CC], F32, tag="V")
            I = pio.tile([P, CC], U32, tag="I")
            v.max(V[:, 0:8], mxc[:])
            v.max_index(I[:, 0:8], V[:, 0:8], mxc[:])
            mxc2 = pio.tile([P, F], F32, tag="mxc2")
            v.match_replace(mxc2[:], V[:, 0:8], mxc[:], MR_IMM)
            v.max(V[:, 8:16], mxc2[:])
            v.max_index(I[:, 8:16], V[:, 8:16], mxc2[:])

            # ---------- global candidate ranks (vs top-8 pool) + v10 ----------
            vdr = pdr.tile([P, 8], F32, tag="vdr")
            nc.sync.dma_start(vdr[:], V[:, 0:8])
            vpool = pio.tile([P, P * 8], F32, tag="pool8")
            nc.sync.dma_start(
                vpool[:],
                vdr[:].rearrange("p j -> (p j)")[None, :].broadcast_to([P, P * 8]))
            rnk = pio.tile([P, CC], F32, tag="rnk")
            for j in range(12):
                scr = plp.tile([P, P * 8], F32, tag="scrj")
                v.tensor_scalar(scr[:], vpool[:], V[:, j:j + 1], None,
                                op0=Alu.is_gt, op1=Alu.add, accum_out=rnk[:, j:j + 1])
            oh10 = pio.tile([P, CC], F32, tag="oh10")
            v.tensor_scalar(oh10[:, 0:12], rnk[:, 0:12], 9.0, None, op0=Alu.is_equal)
            pv2 = pio.tile([P, 2], F32, tag="pv2")
            scrd = pio.tile([P, 16], F32, tag="scrd")
            v.scalar_tensor_tensor(scrd[:, 0:12], oh10[:, 0:12], 1.0, V[:, 0:12],
                                   op0=Alu.mult, op1=Alu.mult, accum_out=pv2[:, 0:1])

            # ---------- neg selection threshold ----------
            vneg = pio.tile([P, F], F32, tag="vneg")
            t2 = pio.tile([P, F], F32, tag="t2")
            v.tensor_scalar(t2[:], negm[:], 2.0, -2.0, op0=Alu.mult, op1=Alu.add)
            v.tensor_tensor(vneg[:], t2[:], sc[:], op=Alu.subtract)
            W = pio.tile([P, NW], F32, tag="W")
            v.max(W[:], vneg[:])
            wdr = pdr.tile([P, NW], F32, tag="wdr")
            nc.sync.dma_start(wdr[:], W[:])
            wpool = pio.tile([P, P * NW], F32, tag="pool8")
            nc.sync.dma_start(
                wpool[:],
                wdr[:].rearrange("p j -> (p j)")[None, :].broadcast_to([P, P * NW]))
            wr = pio.tile([P, NW], F32, tag="wr")
            for j in range(NW):
                wscr = plp.tile([P, P * 8], F32, tag="scrj")
                v.tensor_scalar(wscr[:, 0:P * NW], wpool[:], W[:, j:j + 1], None,
                                op0=Alu.is_gt, op1=Alu.add, accum_out=wr[:, j:j + 1])
            km1 = pio.tile([P, 1], F32, tag="km1")
            v.tensor_scalar(km1[:], kk[:], -1.0, None, op0=Alu.add)
            ohw = pio.tile([P, NW], F32, tag="ohw")
            v.tensor_scalar(ohw[:], wr[:], km1[:], None, op0=Alu.is_equal)
            v.scalar_tensor_tensor(scrd[:, 0:NW], ohw[:], 1.0, W[:],
                                   op0=Alu.mult, op1=Alu.mult, accum_out=pv2[:, 1:2])
            pv2r = pio.tile([P, 2], F32, tag="pv2r")
            psum_bcast(pv2r[:], pv2[:], 2)
            v10 = pv2r[:, 0:1]
            tauv = pv2r[:, 1:2]
            taup = pio.tile([P, 1], F32, tag="taup")
            v.tensor_scalar(t1[:], v10, -POS_T, None, op0=Alu.add)
            v.tensor_tensor(t1[:], t1[:], use_fb[:], op=Alu.mult)
            v.tensor_scalar(taup[:], t1[:], POS_T, None, op0=Alu.add)

            # ---------- dense neg focal ----------
            ce_n = pio.tile([P, F], F32, tag="cen")
            v.tensor_tensor(ce_n[:], lse, cls[:, 0, :], op=Alu.subtract)
            pt_n = pio.tile([P, F], F32, tag="ptn")
            s.activation(pt_n[:], ce_n[:], Act.Exp, scale=-1.0)
            u_n = pio.tile([P, F], F32, tag="un2")
            v.tensor_scalar(u_n[:], pt_n[:], -1.0, 1.0, op0=Alu.mult, op1=Alu.add)
            u2_n = pio.tile([P, F], F32, tag="u2n")
            s.activation(u2_n[:], u_n[:], Act.Square)
            foc_n = pio.tile([P, F], F32, tag="focn")
            v.scalar_tensor_tensor(foc_n[:], u2_n[:], ALPHA, ce_n[:],
                                   op0=Alu.mult, op1=Alu.mult)
            sums = pio.tile([P, 4], F32, tag="sums")
            selm = pio.tile([P, F], F32, tag="selm")
            v.tensor_scalar(selm[:], vneg[:], tauv, None, op0=Alu.is_ge)
            v.scalar_tensor_tensor(selm[:], selm[:], 1.0, foc_n[:],
                                   op0=Alu.mult, op1=Alu.mult, accum_out=sums[:, 0:1])
            allm = pio.tile([P, F], F32, tag="allm")
            v.scalar_tensor_tensor(allm[:], negm[:], 1.0, foc_n[:],
                                   op0=Alu.mult, op1=Alu.mult, accum_out=sums[:, 1:2])

            # ---------- route top-128 candidates to partition slots via PE ----------
            If = pio.tile([P, CC], F32, tag="If")
            v.tensor_copy(If[:], I[:])
            OH1 = pio.tile([P, P], F32, tag="OH1")
            Wf = pio.tile([P, P], F32, tag="Wf")
            v.memset(OH1[:], 0.0)
            v.memset(Wf[:], 0.0)
            for j in range(12):
                ohj = plp.tile([P, P], F32, tag="ohj")
                v.tensor_scalar(ohj[:], iota_r[:], rnk[:, j:j + 1], None,
                                op0=Alu.is_equal)
                v.tensor_tensor(OH1[:], OH1[:], ohj[:], op=Alu.add)
                v.scalar_tensor_tensor(Wf[:], ohj[:], If[:, j:j + 1], Wf[:],
                                       op0=Alu.mult, op1=Alu.add)
            # fsel[r] = f-index of rank-r candidate (column sums of Wf)
            psF = pps.tile([1, P], F32, tag="psF")
            nc.tensor.matmul(psF[:], ones[:], Wf[:])
            rowF = pio.tile([1, P], F32, tag="rowF")
            v.tensor_copy(rowF[:], psF[:])
            dF = pdr.tile([1, P], F32, tag="dF")
            nc.sync.dma_start(dF[:], rowF[:])
            fsel = pio.tile([P, 1], F32, tag="fsel")
            nc.sync.dma_start(
                fsel[:], dF[:].rearrange("o p -> (o p)").rearrange("(p o) -> p o", o=1))
            # stage A: permute payload rows (candidate r -> partition r)
            srcs = (mx[:], dx0, dx1, dy0, dy1, An, lse)
            ohf = pio.tile([P, F], F32, tag="ohf")
            v.tensor_scalar(ohf[:], iota_f[:], fsel[:], None, op0=Alu.is_equal)
            cX = pio.tile([P, 8], F32, tag="cX")
            junk2 = pio.tile([P, F], F32, tag="junkF")
            for ci, sap in enumerate(srcs):
                psA = pps.tile([P, F], F32, tag="psA")
                nc.tensor.matmul(psA[:], OH1[:], sap)
                prm = plp.tile([P, F], F32, tag="prm")
                s.activation(prm[:], psA[:], Act.Copy)
                v.scalar_tensor_tensor(junk2[:], ohf[:], 1.0, prm[:],
                                       op0=Alu.mult, op1=Alu.mult,
                                       accum_out=cX[:, ci:ci + 1])
            cV = cX[:, 0:1]
            cdx0 = cX[:, 1:2]
            cdx1 = cX[:, 2:3]
            cdy0 = cX[:, 3:4]
            cdy1 = cX[:, 4:5]
            cAn = cX[:, 5:6]
            clse = cX[:, 6:7]

            # ---------- candidate iou vs all 20 gts -> first argmax ----------
            q0 = pio.tile([P, M], F32, tag="q0")
            q1 = pio.tile([P, M], F32, tag="q1")
            iwm = pio.tile([P, M], F32, tag="iwm")
            iom = pio.tile([P, M], F32, tag="iom2")
            v.tensor_scalar(q0[:], bx0, cdx0, None, op0=Alu.max)
            v.tensor_scalar(q1[:], bx1, cdx1, None, op0=Alu.min)
            v.tensor_tensor(q1[:], q1[:], q0[:], op=Alu.subtract)
            v.tensor_scalar(iwm[:], q1[:], 0.0, None, op0=Alu.max)
            v.tensor_scalar(q0[:], by0, cdy0, None, op0=Alu.max)
            v.tensor_scalar(q1[:], by1, cdy1, None, op0=Alu.min)
            v.tensor_tensor(q1[:], q1[:], q0[:], op=Alu.subtract)
            v.tensor_scalar(q1[:], q1[:], 0.0, None, op0=Alu.max)
            v.tensor_tensor(iom[:], iwm[:], q1[:], op=Alu.mult)     # inter
            v.tensor_scalar(q0[:], bA[:], cAn, None, op0=Alu.add)   # S = An+Am
            v.reciprocal(q0[:], q0[:])
            v.tensor_tensor(iom[:], iom[:], q0[:], op=Alu.mult)     # r
            eqm = pio.tile([P, M], F32, tag="eqm")
            # tolerance match: |iou_c - mx| <= 1e-6 (bit-exact equality is
            # fragile across ACT-relu vs DVE-max rounding on HW)
            v.tensor_scalar(eqm[:], iom[:], cV, None, op0=Alu.subtract)
            v.tensor_tensor(eqm[:], eqm[:], eqm[:], op=Alu.mult)
            v.tensor_scalar(eqm[:], eqm[:], 1.0e-12, None, op0=Alu.is_le)
            v.scalar_tensor_tensor(eqm[:], eqm[:], -999.0, iota_m[:],
                                   op0=Alu.mult, op1=Alu.add)
            mstar = pio.tile([P, 1], F32, tag="mstar")
            v.tensor_reduce(mstar[:], eqm[:], axis=mybir.AxisListType.X, op=Alu.min)
            v.tensor_scalar(mstar[:], mstar[:], 999.0, None, op0=Alu.add)
            v.tensor_scalar(mstar[:], mstar[:], float(M - 1), None, op0=Alu.min)
            ohm = pio.tile([P, M], F32, tag="ohm")
            v.tensor_scalar(ohm[:], iota_m[:], mstar[:], None, op0=Alu.is_equal)
            cgt = pio.tile([P, 8], F32, tag="cgt")
            for gi, gap in enumerate((bx0, by0, bx1, by1, bA[:], tlf[:])):
                gjunk = plp.tile([P, M], F32, tag="gjunk")
                v.scalar_tensor_tensor(gjunk[:], ohm[:], 1.0, gap,
                                       op0=Alu.mult, op1=Alu.mult,
                                       accum_out=cgt[:, gi:gi + 1])
            cbx0 = cgt[:, 0:1]
            cby0 = cgt[:, 1:2]
            cbx1 = cgt[:, 2:3]
            cby1 = cgt[:, 3:4]
            cbA = cgt[:, 4:5]
            ctl = cgt[:, 5:6]

            # ---------- candidate cls value via class-masked accumulated permute ----------
            dtg = pdr.tile([1, P], F32, tag="dtg")
            nc.sync.dma_start(
                dtg[:].rearrange("o p -> (o p)").rearrange("(p o) -> p o", o=1), ctl)
            tgrow = pio.tile([P, P], F32, tag="tgrow")
            nc.sync.dma_start(tgrow[:], dtg[:].broadcast_to([P, P]))
            psC = pps.tile([P, F], F32, tag="psC")
            for c in range(C):
                ohct = plp.tile([P, P], F32, tag="ohct")
                v.tensor_scalar(ohct[:], tgrow[:], float(c), None, op0=Alu.is_equal)
                v.tensor_tensor(ohct[:], ohct[:], OH1[:], op=Alu.mult)
                nc.tensor.matmul(psC[:], ohct[:], cls[:, c, :],
                                 start=(c == 0), stop=(c == C - 1))
            clsPick = pio.tile([P, F], F32, tag="mxc2")
            s.activation(clsPick[:], psC[:], Act.Copy)
            ccls = pio.tile([P, 1], F32, tag="ccls")
            v.scalar_tensor_tensor(junk2[:], ohf[:], 1.0, clsPick[:],
                                   op0=Alu.mult, op1=Alu.mult, accum_out=ccls[:])

            # ---------- candidate pos focal ----------
            posf = pio.tile([P, 1], F32, tag="posf")
            v.tensor_scalar(posf[:], cV, taup[:], None, op0=Alu.is_ge)
            ce_p = pio.tile([P, 1], F32, tag="cep")
            v.tensor_tensor(ce_p[:], clse, ccls[:], op=Alu.subtract)
            pt_p = pio.tile([P, 1], F32, tag="ptp")
            s.activation(pt_p[:], ce_p[:], Act.Exp, scale=-1.0)
            u_p = pio.tile([P, 1], F32, tag="up")
            v.tensor_scalar(u_p[:], pt_p[:], -1.0, 1.0, op0=Alu.mult, op1=Alu.add)
            v.tensor_tensor(u_p[:], u_p[:], u_p[:], op=Alu.mult)
            foc_p = pio.tile([P, 1], F32, tag="focp")
            v.scalar_tensor_tensor(foc_p[:], u_p[:], ALPHA, ce_p[:],
                                   op0=Alu.mult, op1=Alu.mult)
            v.tensor_tensor(sums[:, 2:3], posf[:], foc_p[:], op=Alu.mult)

            # ---------- candidate giou ----------
            onemv = pio.tile([P, 1], F32, tag="onemv")      # 1 - V
            v.tensor_scalar(onemv[:], cV, -1.0, 1.0, op0=Alu.mult, op1=Alu.add)
            cun = pio.tile([P, 1], F32, tag="cun")
            ctt = pio.tile([P, 1], F32, tag="ctt")
            v.tensor_tensor(ctt[:], cAn, cbA, op=Alu.add)
            v.tensor_tensor(cun[:], onemv[:], ctt[:], op=Alu.mult)  # union = S*(1-V)
            iouv = pio.tile([P, 1], F32, tag="iouv")        # true iou = V/(1-V)
            v.reciprocal(iouv[:], onemv[:])
            v.tensor_tensor(iouv[:], iouv[:], cV, op=Alu.mult)
            ce0 = pio.tile([P, 1], F32, tag="ce0")
            ce1 = pio.tile([P, 1], F32, tag="ce1")
            cf0 = pio.tile([P, 1], F32, tag="cf0")
            cf1 = pio.tile([P, 1], F32, tag="cf1")
            v.tensor_tensor(ce0[:], cdx0, cbx0, op=Alu.min)
            v.tensor_tensor(ce1[:], cdx1, cbx1, op=Alu.max)
            v.tensor_tensor(ce1[:], ce1[:], ce0[:], op=Alu.subtract)
            v.tensor_tensor(cf0[:], cdy0, cby0, op=Alu.min)
            v.tensor_tensor(cf1[:], cdy1, cby1, op=Alu.max)
            v.tensor_tensor(cf1[:], cf1[:], cf0[:], op=Alu.subtract)
            cenc = pio.tile([P, 1], F32, tag="cenc")
            v.tensor_tensor(cenc[:], ce1[:], cf1[:], op=Alu.mult)
            cre = pio.tile([P, 1], F32, tag="cre")
            v.reciprocal(cre[:], cenc[:])
            v.tensor_tensor(cenc[:], cenc[:], cun[:], op=Alu.subtract)
            v.tensor_tensor(cenc[:], cenc[:], cre[:], op=Alu.mult)
            cgi = pio.tile([P, 1], F32, tag="cgi")
            v.tensor_tensor(cgi[:], iouv[:], cenc[:], op=Alu.subtract)
            v.tensor_scalar(cgi[:], cgi[:], -1.0, 1.0, op0=Alu.mult, op1=Alu.add)
            v.tensor_tensor(sums[:, 3:4], posf[:], cgi[:], op=Alu.mult)

            sumr = pio.tile([P, 4], F32, tag="sumr")
            psum_bcast(sumr[:], sums[:], 4)
            sel_sum = sumr[:, 0:1]
            allneg_sum = sumr[:, 1:2]
            pos_sum = sumr[:, 2:3]
            reg_sum = sumr[:, 3:4]

            # ---------- combine ----------
            branch = pio.tile([P, 1], F32, tag="branch")   # nneg > k
            v.tensor_scalar(branch[:], nneg, kk[:], None, op0=Alu.is_gt)
            negsum = pio.tile([P, 1], F32, tag="negsum")
            v.tensor_tensor(t1[:], sel_sum, allneg_sum, op=Alu.subtract)
            v.tensor_tensor(t1[:], t1[:], branch[:], op=Alu.mult)
            v.tensor_tensor(negsum[:], allneg_sum, t1[:], op=Alu.add)
            negcnt = pio.tile([P, 1], F32, tag="negcnt")
            v.tensor_tensor(t1[:], kk[:], nneg, op=Alu.subtract)
            v.tensor_tensor(t1[:], t1[:], branch[:], op=Alu.mult)
            v.tensor_tensor(negcnt[:], nneg, t1[:], op=Alu.add)
            tots = pio.tile([P, 1], F32, tag="tots")
            v.tensor_tensor(tots[:], num_pos[:], negcnt[:], op=Alu.add)
            v.tensor_scalar(tots[:], tots[:], 1.0, None, op0=Alu.max)
            v.reciprocal(tots[:], tots[:])
            clsl = pio.tile([P, 1], F32, tag="clsl")
            v.tensor_tensor(clsl[:], pos_sum, negsum[:], op=Alu.add)
            v.tensor_tensor(clsl[:], clsl[:], tots[:], op=Alu.mult)
            npc = pio.tile([P, 1], F32, tag="npc")
            v.tensor_scalar(npc[:], num_pos[:], 1.0, None, op0=Alu.max)
            v.reciprocal(npc[:], npc[:])
            regl = pio.tile([P, 1], F32, tag="regl")
            v.tensor_tensor(regl[:], reg_sum, npc[:], op=Alu.mult)
            v.tensor_tensor(clsl[:], clsl[:], regl[:], op=Alu.add)
            v.tensor_tensor(acc_part[:], acc_part[:], clsl[:], op=Alu.add)

        nc.sync.dma_start(o_part[:], acc_part[:1, 0:1])


# ======================= host-side runner =======================
_CACHE = {}


def _split_multiwaits(bj):
    """This container's walrus supports one sync-wait per instruction; split
    Tile's multi-wait instructions into NoOp chains at BIR-JSON level."""
    import json
    m = json.loads(bj)
    for fn in m["functions"]:
        for b in fn["blocks"]:
            out = []
            for i in b.get("instructions", []):
                si = i.get("sync_info") or {}
                ow = si.get("on_wait") or []
                if len(ow) > 1:
                    for w_ix, w in enumerate(ow[:-1]):
                        out.append({"name": f"{i['name']}_w{w_ix}",
                                    "opcode": "NoOp", "engine": i["engine"],
                                    "ins": [], "outs": [],
                                    "sync_info": {"on_wait": [w],
                                                  "on_update": []}})
                    si["on_wait"] = [ow[-1]]
                out.append(i)
            b["instructions"] = out
    return json.dumps(m).encode()


def _install_bir_patch():
    import concourse.bass2jax as b2j
    if getattr(b2j, "_mw_patched", False):
        return
    orig = b2j.compile_bir_kernel

    def patched(bir_json, tmpdir, neff_name="file.neff"):
        return orig(_split_multiwaits(bir_json), tmpdir, neff_name=neff_name)

    b2j.compile_bir_kernel = patched
    b2j._mw_patched = True


def _get_nc():
    if "nc" in _CACHE:
        return _CACHE["nc"]
    import concourse.tile as tile
    nc = bass.Bass("TRN2", target_bir_lowering=False, debug=False)
    d_cls = nc.dram_tensor("d_cls", [IMGS, C, N], F32, kind="ExternalInput").ap()
    d_reg = nc.dram_tensor("d_reg", [IMGS, 4, N], F32, kind="ExternalInput").ap()
    d_anch = nc.dram_tensor("d_anch", [N * 4], F32, kind="ExternalInput").ap()
    d_tb = nc.dram_tensor("d_tb", [IMGS, M, 4], F32, kind="ExternalInput").ap()
    d_tl = nc.dram_tensor("d_tl", [IMGS, M], I32, kind="ExternalInput").ap()
    d_sc = nc.dram_tensor("d_sc", [IMGS, N], F32, kind="ExternalInput").ap()
    d_out = nc.dram_tensor("d_out", [1, 1], F32, kind="ExternalOutput").ap()
    with tile.TileContext(nc) as tc:
        build(nc, tc, [d_out], [d_cls, d_reg, d_anch, d_tb, d_tl, d_sc])
    _CACHE["nc"] = nc
    return nc


def _in_maps(cls_output, reg_output, anchors, target_boxes, target_labels,
             neg_scores, n_cores=8):
    B = cls_output.shape[0]
    assert B == n_cores * IMGS
    maps = []
    for cix in range(n_cores):
        i0 = cix * IMGS
        sl = slice(i0, i0 + IMGS)
        maps.append({
            "d_cls": np.ascontiguousarray(
                np.asarray(cls_output[sl], np.float32).reshape(IMGS, C, N)),
            "d_reg": np.ascontiguousarray(
                np.asarray(reg_output[sl], np.float32).reshape(IMGS, 4, N)),
            "d_anch": np.ascontiguousarray(
                np.asarray(anchors, np.float32).reshape(N * 4)),
            "d_tb": np.ascontiguousarray(
                np.asarray(target_boxes[sl], np.float32)),
            "d_tl": np.ascontiguousarray(
                np.asarray(target_labels[sl]).astype(np.int32)),
            "d_sc": np.ascontiguousarray(
                np.asarray(neg_scores[sl], np.float32)),
        })
    return maps


def kernel(cls_output, reg_output, anchors, target_boxes, target_labels,
           neg_scores):
    from concourse.bass_utils import run_bass_kernel_spmd
    _install_bir_patch()
    nc = _get_nc()
    maps = _in_maps(cls_output, reg_output, anchors, target_boxes,
                    target_labels, neg_scores)
    res = run_bass_kernel_spmd(nc, maps, core_ids=list(range(8)))
    B = cls_output.shape[0]
    total = sum(float(r["d_out"][0, 0]) for r in res.results) / B
    return np.array(total, dtype=np.float32)



# revision 22
# speedup vs baseline: 2.1261x; 2.1261x over previous
"""Detection-loss Bass kernel, v2.

Per core: 2 images. Strategy:
 - bf16 *proxy* dense phase: decode + max-IoU (r-space) over J=6
   per-partition-pruned boxes; used only for (a) neg-mask sign,
   (b) per-partition top-12 candidate selection.
 - exact f32 refinement for the 12 candidates/partition: payload
   (reg/anchor/cls/lse/gt-box/label) fetched via indirect-DMA gathers
   from DRAM, decode + IoU vs all 20 boxes recomputed exactly.
 - LSE: streamed cls chunks -> ACT exp (bf16) -> PE identity-matmul
   accumulation into PSUM (f32) -> ln.
 - cross-partition reductions/broadcasts via PE matmuls (no DRAM
   roundtrips except tiny pool spills).
Host divides the 8 per-core partial sums by B=16.

Container constraints: single sync-wait walrus (NoOp split patch), no
extended q7 ops, no custom DVE ops.
"""
import numpy as np
import concourse.bass as bass
import concourse.mybir as mybir

F32 = mybir.dt.float32
BF16 = mybir.dt.bfloat16
I32 = mybir.dt.int32
U32 = mybir.dt.uint32
Alu = mybir.AluOpType
Act = mybir.ActivationFunctionType

P = 128
F = 512
N = P * F          # 65536
C = 21
M = 20
J = 6              # pruned boxes per partition for the proxy loop
CC = 12            # candidates per partition
IMGS = 2
MR_IMM = -1.0e30

POS_T = 0.5                  # iou-space positive threshold
NEG_R = 0.4 / 1.4            # r-space negative threshold
ALPHA = 0.25
MIN_POS = 10.0
RATIO = 3.0


DEBUG = False
_DBG_SPECS = []   # (name, shape, dtype) filled at build when DEBUG


def build(nc, tc, outs, ins):
    v = nc.vector
    g = nc.gpsimd
    s = nc.scalar
    o_part = outs[0]
    dbg_outs = outs[1:]
    _dbg_ix = [0]

    def tap(name, ap):
        if not DEBUG:
            return
        nc.sync.dma_start(dbg_outs[_dbg_ix[0]][:], ap)
        _dbg_ix[0] += 1

    d_cls, d_reg, d_anch, d_tb, d_tl, d_sc = ins

    with tc.tile_pool(name="main", bufs=1) as pl, \
         tc.tile_pool(name="io", bufs=1) as pio, \
         tc.tile_pool(name="lp", bufs=2) as plp, \
         tc.tile_pool(name="ps", bufs=1, space="PSUM") as pps, \
         tc.tile_pool(name="dr", bufs=1, space="DRAM") as pdr:

        # ================= static prep (once per core) =================
        anch = pl.tile([P, 2048], F32, tag="anch")
        nc.sync.dma_start(anch[:], d_anch.rearrange("(p f) -> p f", p=P))
        a0 = anch[:, 0:2048:4]
        a1 = anch[:, 1:2048:4]
        a2 = anch[:, 2:2048:4]
        a3 = anch[:, 3:2048:4]
        awf = pl.tile([P, F], F32, tag="awf")
        ahf = pl.tile([P, F], F32, tag="ahf")
        axf = pl.tile([P, F], F32, tag="axf")
        ayf = pl.tile([P, F], F32, tag="ayf")
        v.tensor_tensor(awf[:], a2, a0, op=Alu.subtract)
        v.tensor_tensor(ahf[:], a3, a1, op=Alu.subtract)
        v.scalar_tensor_tensor(axf[:], awf[:], 0.5, a0, op0=Alu.mult, op1=Alu.add)
        v.scalar_tensor_tensor(ayf[:], ahf[:], 0.5, a1, op0=Alu.mult, op1=Alu.add)
        awb = pl.tile([P, F], BF16, tag="awb")
        ahb = pl.tile([P, F], BF16, tag="ahb")
        axb = pl.tile([P, F], BF16, tag="axb")
        ayb = pl.tile([P, F], BF16, tag="ayb")
        v.tensor_copy(awb[:], awf[:])
        v.tensor_copy(ahb[:], ahf[:])
        v.tensor_copy(axb[:], axf[:])
        v.tensor_copy(ayb[:], ayf[:])

        ones = pl.tile([P, 1], F32, tag="ones")
        v.memset(ones[:], 1.0)
        ones1p = pl.tile([1, P], F32, tag="ones1p")
        v.memset(ones1p[:], 1.0)
        iota_m_i = pl.tile([P, M], I32, tag="iomi")
        g.iota(iota_m_i[:], pattern=[[1, M]], base=0, channel_multiplier=0)
        iota_m = pl.tile([P, M], F32, tag="iom")
        v.tensor_copy(iota_m[:], iota_m_i[:])
        iota_p_i = pl.tile([P, 1], I32, tag="iopi")
        g.iota(iota_p_i[:], pattern=[[0, 1]], base=0, channel_multiplier=1)
        iota_pf = pl.tile([P, 1], F32, tag="iopf")        # value p
        v.tensor_copy(iota_pf[:], iota_p_i[:])
        iota_r_i = pl.tile([P, P], I32, tag="iori")
        g.iota(iota_r_i[:], pattern=[[1, P]], base=0, channel_multiplier=0)
        iota_r = pl.tile([P, P], F32, tag="ior")
        v.tensor_copy(iota_r[:], iota_r_i[:])
        # identity matrices
        idf = pl.tile([P, P], F32, tag="idf")             # f32 identity
        v.tensor_scalar(idf[:], iota_r[:], iota_pf[:], None, op0=Alu.is_equal)
        idb = pl.tile([P, P], BF16, tag="idb")            # bf16 identity
        v.tensor_copy(idb[:], idf[:])

        acc_part = pl.tile([1, 1], F32, tag="accp")
        v.memset(acc_part[:], 0.0)

        def colsum(dst_ps, src):
            """[P, n] -> [1, n] column sums in PSUM (dst_ps)."""
            nc.tensor.matmul(dst_ps, ones[:], src)

        def bcast_row(dst_ps, row):
            """[1, n] -> [P, n] broadcast in PSUM (dst_ps)."""
            nc.tensor.matmul(dst_ps, ones1p[:], row)

        for img in range(IMGS):
            t = f"i{img}_"

            # ================= loads =================
            regs = pio.tile([P, 4, F], F32, tag="regs")
            nc.sync.dma_start(
                regs[:], d_reg[img, :, :].rearrange("r (p f) -> p r f", p=P))
            sc = pio.tile([P, F], F32, tag=t + "sc")
            nc.sync.dma_start(sc[:], d_sc[img, :].rearrange("(p f) -> p f", p=P))
            bgt = pio.tile([P, 80], F32, tag=t + "bgt")
            nc.sync.dma_start(
                bgt[:],
                d_tb[img, :, :].rearrange("m c -> (m c)")[None, :].broadcast_to([P, 80]))
            bx0 = bgt[:, 0:80:4]
            by0 = bgt[:, 1:80:4]
            bx1 = bgt[:, 2:80:4]
            by1 = bgt[:, 3:80:4]
            tli = pio.tile([P, M], I32, tag=t + "tli")
            nc.sync.dma_start(
                tli[:], d_tl[img, :][None, :].broadcast_to([P, M]))
            tlf = pio.tile([P, M], F32, tag=t + "tlf")
            v.tensor_copy(tlf[:], tli[:])
            bA = pio.tile([P, M], F32, tag=t + "bA")
            bwt = pio.tile([P, M], F32, tag=t + "bw")
            v.tensor_tensor(bwt[:], bx1, bx0, op=Alu.subtract)
            v.tensor_tensor(bA[:], by1, by0, op=Alu.subtract)
            v.tensor_tensor(bA[:], bA[:], bwt[:], op=Alu.mult)

            # ================= LSE: stream cls chunks =================
            cls0 = pio.tile([P, F], F32, tag=t + "cls0")
            ps_es = pps.tile([P, F], F32, tag="pses")
            for ci in range(7):
                c0 = 3 * ci
                chunk = plp.tile([P, 3, F], F32, tag="chk")
                nc.sync.dma_start(
                    chunk[:],
                    d_cls[img, c0:c0 + 3, :].rearrange("c (p f) -> p c f", p=P))
                if ci == 0:
                    v.tensor_copy(cls0[:], chunk[:, 0, :])
                echk = plp.tile([P, 3, F], BF16, tag="echk")
                s.activation(echk[:], chunk[:], Act.Exp)
                for cc in range(3):
                    nc.tensor.matmul(ps_es[:], idb[:], echk[:, cc, :],
                                     start=(c0 + cc == 0), stop=(c0 + cc == C - 1))
            esum = pio.tile([P, F], F32, tag="esum")
            v.tensor_copy(esum[:], ps_es[:])
            lse = pio.tile([P, F], F32, tag=t + "lse")
            s.activation(lse[:], esum[:], Act.Ln)

            # ================= proxy decode (bf16) =================
            regsb = pio.tile([P, 4, F], BF16, tag="regsb")
            v.tensor_copy(regsb[:], regs[:])
            ewb = pio.tile([P, F], BF16, tag="ewb")
            ehb = pio.tile([P, F], BF16, tag="ehb")
            s.activation(ewb[:], regsb[:, 2, :], Act.Exp)
            s.activation(ehb[:], regsb[:, 3, :], Act.Exp)
            wb = pio.tile([P, F], BF16, tag="wb")
            hb = pio.tile([P, F], BF16, tag="hb")
            v.tensor_tensor(wb[:], awb[:], ewb[:], op=Alu.mult)
            g.tensor_tensor(hb[:], ahb[:], ehb[:], op=Alu.mult)
            cxb = pio.tile([P, F], BF16, tag="cxb")
            cyb = pio.tile([P, F], BF16, tag="cyb")
            v.tensor_tensor(cxb[:], regsb[:, 0, :], awb[:], op=Alu.mult)
            v.tensor_tensor(cxb[:], cxb[:], axb[:], op=Alu.add)
            g.tensor_tensor(cyb[:], regsb[:, 1, :], ahb[:], op=Alu.mult)
            g.tensor_tensor(cyb[:], cyb[:], ayb[:], op=Alu.add)
            w2 = pio.tile([P, F], BF16, tag="w2")
            h2 = pio.tile([P, F], BF16, tag="h2")
            v.tensor_scalar(w2[:], wb[:], 0.5, None, op0=Alu.mult)
            v.tensor_scalar(h2[:], hb[:], 0.5, None, op0=Alu.mult)
            dx0 = pio.tile([P, F], BF16, tag="dx0")
            dx1 = pio.tile([P, F], BF16, tag="dx1")
            dy0 = pio.tile([P, F], BF16, tag="dy0")
            dy1 = pio.tile([P, F], BF16, tag="dy1")
            v.tensor_tensor(dx0[:], cxb[:], w2[:], op=Alu.subtract)
            v.tensor_tensor(dx1[:], cxb[:], w2[:], op=Alu.add)
            g.tensor_tensor(dy0[:], cyb[:], h2[:], op=Alu.subtract)
            g.tensor_tensor(dy1[:], cyb[:], h2[:], op=Alu.add)
            Anb = pio.tile([P, F], BF16, tag="Anb")
            v.tensor_tensor(Anb[:], wb[:], hb[:], op=Alu.mult)

            # ================= payload staging (n-major) =================
            pay3 = pio.tile([P, F, 12], F32, tag="pay3")
            v.tensor_copy(pay3[:, :, 0], regs[:, 0, :])
            v.tensor_copy(pay3[:, :, 1], regs[:, 1, :])
            v.tensor_copy(pay3[:, :, 2], regs[:, 2, :])
            v.tensor_copy(pay3[:, :, 3], regs[:, 3, :])
            v.tensor_copy(pay3[:, :, 4], awf[:])
            v.tensor_copy(pay3[:, :, 5], ahf[:])
            v.tensor_copy(pay3[:, :, 6], axf[:])
            v.tensor_copy(pay3[:, :, 7], ayf[:])
            v.tensor_copy(pay3[:, :, 8], lse[:])
            dpay = pdr.tile([P, F, 12], F32, tag=t + "dpay")
            nc.sync.dma_start(dpay[:], pay3[:])
            probe = pio.tile([P, 1], F32, tag=t + "probe")
            nc.sync.dma_start(probe[:], dpay[:, 0, 0:1])

            # ================= per-partition box pruning =================
            xlo = pio.tile([P, 1], F32, tag=t + "xlo")
            xhi = pio.tile([P, 1], F32, tag=t + "xhi")
            ylo = pio.tile([P, 1], F32, tag=t + "ylo")
            yhi = pio.tile([P, 1], F32, tag=t + "yhi")
            amn = pio.tile([P, 1], F32, tag=t + "amn")
            v.tensor_reduce(xlo[:], dx0[:], axis=mybir.AxisListType.X, op=Alu.min)
            v.tensor_reduce(xhi[:], dx1[:], axis=mybir.AxisListType.X, op=Alu.max)
            v.tensor_reduce(ylo[:], dy0[:], axis=mybir.AxisListType.X, op=Alu.min)
            v.tensor_reduce(yhi[:], dy1[:], axis=mybir.AxisListType.X, op=Alu.max)
            v.tensor_reduce(amn[:], Anb[:], axis=mybir.AxisListType.X, op=Alu.min)
            # ub over [P, M]
            ub1 = pio.tile([P, M], F32, tag=t + "ub1")
            ub2 = pio.tile([P, M], F32, tag=t + "ub2")
            v.tensor_scalar(ub1[:], bx1, xhi[:], None, op0=Alu.min)
            v.scalar_tensor_tensor(ub1[:], bx0, xlo[:], ub1[:], op0=Alu.max,
                                   op1=Alu.subtract)     # -iwu
            v.tensor_scalar(ub2[:], by1, yhi[:], None, op0=Alu.min)
            v.scalar_tensor_tensor(ub2[:], by0, ylo[:], ub2[:], op0=Alu.max,
                                   op1=Alu.subtract)     # -ihu
            v.tensor_scalar(ub2[:], ub2[:], 0.0, None, op0=Alu.min)  # -relu(ihu)
            ubq = pio.tile([P, M], F32, tag=t + "ubq")
            v.tensor_tensor(ubq[:], ub1[:], ub2[:], op=Alu.mult)  # iwu*relu(ihu)
            ubd = pio.tile([P, M], F32, tag=t + "ubd")
            v.tensor_scalar(ubd[:], bA[:], amn[:], None, op0=Alu.add)
            v.reciprocal(ubd[:], ubd[:])
            v.tensor_tensor(ubq[:], ubq[:], ubd[:], op=Alu.mult)  # ub score
            # top-J boxes by ub
            ubV = pio.tile([P, 8], F32, tag=t + "ubV")
            ubI = pio.tile([P, 8], U32, tag=t + "ubI")
            v.max(ubV[:], ubq[:])
            v.max_index(ubI[:], ubV[:], ubq[:])
            ubIf = pio.tile([P, 8], F32, tag=t + "ubIf")
            v.tensor_copy(ubIf[:], ubI[:])
            # gather J boxes' coords+area: ohall [P, J, M]
            ohJ = pio.tile([P, J, M], F32, tag=t + "ohJ")
            v.tensor_tensor(
                ohJ[:], iota_m[:, None, :].broadcast_to([P, J, M]),
                ubIf[:, 0:J, None].broadcast_to([P, J, M]), op=Alu.is_equal)
            bxJ = pio.tile([P, 5, J], F32, tag=t + "bxJ")
            for ci_, cap in enumerate((bx0, by0, bx1, by1, bA[:])):
                tmpJ = plp.tile([P, J, M], F32, tag="tmpJ")
                v.tensor_tensor(tmpJ[:], ohJ[:],
                                cap[:, None, :].broadcast_to([P, J, M]),
                                op=Alu.mult)
                v.tensor_reduce(bxJ[:, ci_, :], tmpJ[:],
                                axis=mybir.AxisListType.X, op=Alu.add)

            # ================= proxy m-loop over J boxes =================
            mx = pio.tile([P, F], BF16, tag=t + "mx")
            v.memset(mx[:], 0.0)
            for j in range(J):
                jx0 = bxJ[:, 0, j:j + 1]
                jy0 = bxJ[:, 1, j:j + 1]
                jx1 = bxJ[:, 2, j:j + 1]
                jy1 = bxJ[:, 3, j:j + 1]
                jA = bxJ[:, 4, j:j + 1]
                cx1 = plp.tile([P, F], BF16, tag="cx1")
                cx0 = plp.tile([P, F], BF16, tag="cx0")
                iw = plp.tile([P, F], BF16, tag="iw")
                cy1 = plp.tile([P, F], BF16, tag="cy1")
                cy0 = plp.tile([P, F], BF16, tag="cy0")
                ih = plp.tile([P, F], BF16, tag="ih")
                ihc = plp.tile([P, F], BF16, tag="ihc")
                qq = plp.tile([P, F], BF16, tag="qq")
                dd = plp.tile([P, F], BF16, tag="dd")
                v.tensor_scalar(cx1[:], dx1[:], jx1, None, op0=Alu.min)
                v.tensor_scalar(cx0[:], dx0[:], jx0, None, op0=Alu.max)
                g.tensor_tensor(iw[:], cx1[:], cx0[:], op=Alu.subtract)
                v.tensor_scalar(cy1[:], dy1[:], jy1, None, op0=Alu.min)
                v.tensor_scalar(cy0[:], dy0[:], jy0, None, op0=Alu.max)
                g.tensor_tensor(ih[:], cy1[:], cy0[:], op=Alu.subtract)
                s.activation(ihc[:], ih[:], Act.Relu)
                v.tensor_tensor(qq[:], iw[:], ihc[:], op=Alu.mult)
                v.tensor_scalar(dd[:], Anb[:], jA, None, op0=Alu.add)
                with nc.allow_low_precision(reason="bf16 proxy iou"):
                    v.reciprocal(dd[:], dd[:])
                v.tensor_tensor(qq[:], qq[:], dd[:], op=Alu.mult)
                v.tensor_tensor(mx[:], mx[:], qq[:], op=Alu.max)

            tap(t + "mx", mx[:])
            tap(t + "bxJ", bxJ[:])
            # ================= neg mask / count =================
            negm = pio.tile([P, F], F32, tag=t + "negm")
            nnegp = pio.tile([P, 1], F32, tag=t + "nnegp")
            v.tensor_scalar(negm[:], mx[:], NEG_R, None, op0=Alu.is_lt,
                            op1=Alu.add, accum_out=nnegp[:])

            # ================= top-12 candidates per partition =================
            V16 = pio.tile([P, 16], BF16, tag=t + "V16")
            I16 = pio.tile([P, 16], U32, tag=t + "I16")
            mxc2 = pio.tile([P, F], BF16, tag=t + "mxc2")
            v.max(V16[:, 0:8], mx[:])
            v.max_index(I16[:, 0:8], V16[:, 0:8], mx[:])
            v.match_replace(mxc2[:], V16[:, 0:8], mx[:], MR_IMM)
            v.max(V16[:, 8:16], mxc2[:])
            v.max_index(I16[:, 8:16], V16[:, 8:16], mxc2[:])
            If = pio.tile([P, CC], F32, tag=t + "If")
            v.tensor_copy(If[:], I16[:, 0:CC])
            # n = p*512 + f  (exact in f32), as i32 offsets
            nfl = pio.tile([P, CC], F32, tag=t + "nfl")
            pbase = pio.tile([P, 1], F32, tag=t + "pbase")
            v.tensor_scalar(pbase[:], iota_pf[:], 512.0, None, op0=Alu.mult)
            v.tensor_scalar(nfl[:], If[:], pbase[:], None, op0=Alu.add)
            # fold 0*probe into offsets: orders gathers after the spill
            zp = pio.tile([P, 1], F32, tag=t + "zp")
            v.tensor_scalar(zp[:], probe[:], 0.0, None, op0=Alu.mult)
            nflo = pio.tile([P, CC], F32, tag=t + "nflo")
            v.tensor_scalar(nflo[:], nfl[:], zp[:], None, op0=Alu.add)
            nidx = pio.tile([P, CC], I32, tag=t + "nidx")
            v.tensor_copy(nidx[:], nflo[:])

            tap(t + "nfl", nfl[:])
            # ================= indirect gathers (exact payload) =================
            cpay = pio.tile([P, CC, 12], F32, tag=t + "cpay")
            for jj in range(CC):
                ij = plp.tile([P, 1], I32, tag=f"ij{jj}")
                v.tensor_copy(ij[:], nidx[:, jj:jj + 1])
                gj = plp.tile([P, 12], F32, tag=f"gj{jj}")
                g.indirect_dma_start(
                    gj[:], None,
                    dpay[:].rearrange("p f s -> (p f) s"),
                    bass.IndirectOffsetOnAxis(ap=ij[:], axis=0))
                v.tensor_copy(cpay[:, jj, :], gj[:])

            tap(t + "cpay", cpay[:])
            # exact decode of candidates [P, CC]
            caw = cpay[:, :, 4]
            cah = cpay[:, :, 5]
            cax = cpay[:, :, 6]
            cay = cpay[:, :, 7]
            clse = pio.tile([P, CC], F32, tag=t + "clse")
            v.tensor_copy(clse[:], cpay[:, :, 8])
            cew = pio.tile([P, CC], F32, tag=t + "cew")
            ceh = pio.tile([P, CC], F32, tag=t + "ceh")
            s.activation(cew[:], cpay[:, :, 2], Act.Exp)
            s.activation(ceh[:], cpay[:, :, 3], Act.Exp)
            cw = pio.tile([P, CC], F32, tag=t + "cw")
            ch = pio.tile([P, CC], F32, tag=t + "ch")
            v.tensor_tensor(cw[:], caw, cew[:], op=Alu.mult)
            v.tensor_tensor(ch[:], cah, ceh[:], op=Alu.mult)
            ccx = pio.tile([P, CC], F32, tag=t + "ccx")
            ccy = pio.tile([P, CC], F32, tag=t + "ccy")
            v.tensor_tensor(ccx[:], cpay[:, :, 0], caw, op=Alu.mult)
            v.tensor_tensor(ccx[:], ccx[:], cax, op=Alu.add)
            v.tensor_tensor(ccy[:], cpay[:, :, 1], cah, op=Alu.mult)
            v.tensor_tensor(ccy[:], ccy[:], cay, op=Alu.add)
            cdx0 = pio.tile([P, CC], F32, tag=t + "cdx0")
            cdx1 = pio.tile([P, CC], F32, tag=t + "cdx1")
            cdy0 = pio.tile([P, CC], F32, tag=t + "cdy0")
            cdy1 = pio.tile([P, CC], F32, tag=t + "cdy1")
            v.scalar_tensor_tensor(cdx0[:], cw[:], -0.5, ccx[:], op0=Alu.mult, op1=Alu.add)
            v.scalar_tensor_tensor(cdx1[:], cw[:], 0.5, ccx[:], op0=Alu.mult, op1=Alu.add)
            v.scalar_tensor_tensor(cdy0[:], ch[:], -0.5, ccy[:], op0=Alu.mult, op1=Alu.add)
            v.scalar_tensor_tensor(cdy1[:], ch[:], 0.5, ccy[:], op0=Alu.mult, op1=Alu.add)
            cAn = pio.tile([P, CC], F32, tag=t + "cAn")
            v.tensor_tensor(cAn[:], cw[:], ch[:], op=Alu.mult)

            # ================= exact IoU vs all 20 boxes [P, CC, M] ==========
            bxt = pio.tile([P, 4, M], F32, tag=t + "bxt")
            v.tensor_copy(bxt[:, 0, :], bx0)
            v.tensor_copy(bxt[:, 1, :], by0)
            v.tensor_copy(bxt[:, 2, :], bx1)
            v.tensor_copy(bxt[:, 3, :], by1)

            def b3(tile_):
                return tile_[:, :, None].broadcast_to([P, CC, M])

            def bm(c_):
                return bxt[:, c_:c_ + 1, :].broadcast_to([P, CC, M])

            q0 = pio.tile([P, CC, M], F32, tag="q0")
            q1 = pio.tile([P, CC, M], F32, tag="q1")
            v.tensor_tensor(q0[:], b3(cdx1), bm(2), op=Alu.min)
            v.tensor_tensor(q1[:], b3(cdx0), bm(0), op=Alu.max)
            g.tensor_tensor(q0[:], q0[:], q1[:], op=Alu.subtract)
            v.tensor_scalar(q0[:], q0[:], 0.0, None, op0=Alu.max)   # iw
            v.tensor_tensor(q1[:], b3(cdy1), bm(3), op=Alu.min)
            qy = pio.tile([P, CC, M], F32, tag="qy")
            v.tensor_tensor(qy[:], b3(cdy0), bm(1), op=Alu.max)
            g.tensor_tensor(q1[:], q1[:], qy[:], op=Alu.subtract)
            v.tensor_scalar(q1[:], q1[:], 0.0, None, op0=Alu.max)   # ih
            v.tensor_tensor(q0[:], q0[:], q1[:], op=Alu.mult)       # inter
            # union = cAn + bA - inter
            g.tensor_tensor(q1[:], b3(cAn), bA[:, None, :].broadcast_to([P, CC, M]),
                            op=Alu.add)
            v.tensor_tensor(q1[:], q1[:], q0[:], op=Alu.subtract)
            v.tensor_scalar(q1[:], q1[:], 1e-7, None, op0=Alu.max)
            v.reciprocal(q1[:], q1[:])
            v.tensor_tensor(q0[:], q0[:], q1[:], op=Alu.mult)       # iou [P,CC,M]
            cV = pio.tile([P, CC], F32, tag=t + "cV")
            v.tensor_reduce(cV[:], q0[:], axis=mybir.AxisListType.X, op=Alu.max)
            # first argmax
            eqm = pio.tile([P, CC, M], F32, tag="eqm")
            v.tensor_tensor(eqm[:], q0[:], b3(cV), op=Alu.is_ge)
            v.scalar_tensor_tensor(eqm[:], eqm[:], -999.0,
                                   iota_m[:, None, :].broadcast_to([P, CC, M]),
                                   op0=Alu.mult, op1=Alu.add)
            mst = pio.tile([P, CC], F32, tag=t + "mst")
            v.tensor_reduce(mst[:], eqm[:], axis=mybir.AxisListType.X, op=Alu.min)
            v.tensor_scalar(mst[:], mst[:], 999.0, float(M - 1), op0=Alu.add,
                            op1=Alu.min)
            msti = pio.tile([P, CC], I32, tag=t + "msti")
            v.tensor_copy(msti[:], mst[:])

            # matched gt box + label via one-hot reduce from resident tiles
            ohm = pio.tile([P, CC, M], F32, tag="ohm")
            v.tensor_tensor(ohm[:], iota_m[:, None, :].broadcast_to([P, CC, M]),
                            mst[:, :, None].broadcast_to([P, CC, M]),
                            op=Alu.is_equal)
            cgt = pio.tile([P, CC, 4], F32, tag=t + "cgt")
            ohtmp = pio.tile([P, CC, M], F32, tag="ohtmp")
            for ci_ in range(4):
                g.tensor_tensor(ohtmp[:], ohm[:],
                                bxt[:, ci_:ci_ + 1, :].broadcast_to([P, CC, M]),
                                op=Alu.mult)
                v.tensor_reduce(cgt[:, :, ci_], ohtmp[:],
                                axis=mybir.AxisListType.X, op=Alu.add)
            ctl = pio.tile([P, CC], F32, tag=t + "ctl")
            g.tensor_tensor(ohtmp[:], ohm[:],
                            tlf[:, None, :].broadcast_to([P, CC, M]), op=Alu.mult)
            v.tensor_reduce(ctl[:], ohtmp[:], axis=mybir.AxisListType.X, op=Alu.add)
            # cls at (label, n): offset = img*21N + label*N + n
            coff = pio.tile([P, CC], F32, tag=t + "coff")
            v.tensor_scalar(coff[:], ctl[:], float(N), float(img * C * N),
                            op0=Alu.mult, op1=Alu.add)
            v.tensor_tensor(coff[:], coff[:], nfl[:], op=Alu.add)
            coffi = pio.tile([P, CC], I32, tag=t + "coffi")
            v.tensor_copy(coffi[:], coff[:])
            ccls = pio.tile([P, CC], F32, tag=t + "ccls")
            for jj in range(CC):
                cj = plp.tile([P, 1], I32, tag=f"cj{jj}")
                v.tensor_copy(cj[:], coffi[:, jj:jj + 1])
                gcj = plp.tile([P, 1], F32, tag=f"gcj{jj}")
                g.indirect_dma_start(
                    gcj[:], None,
                    d_cls.rearrange("i c (n o) -> (i c n) o", o=1),
                    bass.IndirectOffsetOnAxis(ap=cj[:], axis=0))
                v.tensor_copy(ccls[:, jj:jj + 1], gcj[:])

            tap(t + "cV", cV[:])
            tap(t + "mst", mst[:])
            tap(t + "ctl", ctl[:])
            tap(t + "ccls", ccls[:])
            # ================= npos / v10 / taup =================
            npp = pio.tile([P, 1], F32, tag=t + "npp")
            junk12 = pio.tile([P, CC], F32, tag=t + "junk12")
            v.tensor_scalar(junk12[:], cV[:], POS_T, None, op0=Alu.is_ge,
                            op1=Alu.add, accum_out=npp[:])
            ps_np = pps.tile([1, P], F32, tag="psrow")
            np2 = pio.tile([P, 2], F32, tag=t + "np2")
            v.tensor_copy(np2[:, 0:1], npp[:])
            v.tensor_copy(np2[:, 1:2], nnegp[:])
            colsum(ps_np[:, 0:2], np2[:])
            rownp = pio.tile([1, 2], F32, tag=t + "rownp")
            v.tensor_copy(rownp[:], ps_np[:, 0:2])
            ps_npb = pps.tile([P, P], F32, tag="psbc")
            bcast_row(ps_npb[:, 0:2], rownp[:])
            cnt2 = pio.tile([P, 2], F32, tag=t + "cnt2")
            s.activation(cnt2[:], ps_npb[:, 0:2], Act.Copy)
            npos_raw = cnt2[:, 0:1]
            nneg = cnt2[:, 1:2]

            # v10 cascade: top-8 cV per partition -> [8,128] -> top16 rows
            cV8 = pio.tile([P, 8], F32, tag=t + "cV8")
            v.max(cV8[:], cV[:])
            ps_t = pps.tile([8, P], F32, tag="pst8")
            nc.tensor.matmul(ps_t[:], cV8[:], idf[:])
            rowt = pio.tile([8, P], F32, tag=t + "rowt")
            s.activation(rowt[:], ps_t[:], Act.Copy)
            r8a = pio.tile([8, 8], F32, tag=t + "r8a")
            r8b = pio.tile([8, 8], F32, tag=t + "r8b")
            rres = pio.tile([8, P], F32, tag=t + "rres")
            v.max(r8a[:], rowt[:])
            v.match_replace(rres[:], r8a[:], rowt[:], MR_IMM)
            v.max(r8b[:], rres[:])
            # spill [8,16] -> reload as row [1,128] and column [128,1]
            dpool = pdr.tile([8, 16], F32, tag=t + "dpool")
            nc.sync.dma_start(dpool[:, 0:8], r8a[:])
            nc.sync.dma_start(dpool[:, 8:16], r8b[:])
            p128 = pio.tile([1, P], F32, tag=t + "p128")
            nc.sync.dma_start(
                p128[:], dpool[:].rearrange("a b -> (a b)")[None, :])
            pcol = pio.tile([P, 1], F32, tag=t + "pcol")
            nc.sync.dma_start(
                pcol[:],
                dpool[:].rearrange("a b -> (a b)").rearrange("(p o) -> p o", o=1))
            # broadcast row to [P, 128]; rank of pcol[p] = #(pool > pcol[p])
            ps_pb = pps.tile([P, P], F32, tag="psbc")
            bcast_row(ps_pb[:], p128[:])
            pbc = pio.tile([P, P], F32, tag="pbc")
            s.activation(pbc[:], ps_pb[:], Act.Copy)
            prnk = pio.tile([P, 1], F32, tag=t + "prnk")
            junkp = pio.tile([P, P], F32, tag="junkp")
            v.tensor_scalar(junkp[:], pbc[:], pcol[:], None, op0=Alu.is_gt,
                            op1=Alu.add, accum_out=prnk[:])
            # 10th largest = min over {rank <= 9}
            psel = pio.tile([P, 1], F32, tag=t + "psel")
            v.tensor_scalar(psel[:], prnk[:], MIN_POS - 1.0, None, op0=Alu.is_le)
            v.tensor_scalar(psel[:], psel[:], -1.0, 1.0, op0=Alu.mult, op1=Alu.add)
            v.scalar_tensor_tensor(psel[:], psel[:], 1.0e30, pcol[:],
                                   op0=Alu.mult, op1=Alu.add)
            ps_v1 = pps.tile([1, P], F32, tag="psrow")
            nc.tensor.matmul(ps_v1[:], psel[:], idf[:])
            rowv = pio.tile([1, P], F32, tag=t + "rowv")
            s.activation(rowv[:], ps_v1[:], Act.Copy)
            v10t = pio.tile([1, 1], F32, tag=t + "v10t")
            v.tensor_reduce(v10t[:], rowv[:], axis=mybir.AxisListType.X, op=Alu.min)
            ps_v10 = pps.tile([P, P], F32, tag="psbc")
            bcast_row(ps_v10[:, 0:1], v10t[:])
            v10b = pio.tile([P, 1], F32, tag=t + "v10b")
            s.activation(v10b[:], ps_v10[:, 0:1], Act.Copy)

            use_fb = pio.tile([P, 1], F32, tag=t + "usefb")
            v.tensor_scalar(use_fb[:], npos_raw, MIN_POS, None, op0=Alu.is_lt)
            num_pos = pio.tile([P, 1], F32, tag=t + "numpos")
            tnp = pio.tile([P, 1], F32, tag=t + "tnp")
            v.tensor_scalar(tnp[:], npos_raw, -1.0, MIN_POS, op0=Alu.mult, op1=Alu.add)
            v.tensor_tensor(tnp[:], tnp[:], use_fb[:], op=Alu.mult)
            v.tensor_tensor(num_pos[:], npos_raw, tnp[:], op=Alu.add)
            kk = pio.tile([P, 1], F32, tag=t + "kk")
            v.tensor_scalar(kk[:], num_pos[:], RATIO, None, op0=Alu.mult)
            taup = pio.tile([P, 1], F32, tag=t + "taup")
            v.tensor_scalar(tnp[:], v10b[:], -POS_T, None, op0=Alu.add)
            v.tensor_tensor(tnp[:], tnp[:], use_fb[:], op=Alu.mult)
            v.tensor_scalar(taup[:], tnp[:], POS_T, None, op0=Alu.add)

            tap(t + "cnt2", cnt2[:1, :])
            tap(t + "v10", v10b[:1, :])
            tap(t + "taup", taup[:1, :])
            # ================= pos focal + giou sums =================
            posf = pio.tile([P, CC], F32, tag=t + "posf")
            v.tensor_scalar(posf[:], cV[:], taup[:], None, op0=Alu.is_ge)
            ce_p = pio.tile([P, CC], F32, tag=t + "cep")
            v.tensor_tensor(ce_p[:], clse[:], ccls[:], op=Alu.subtract)
            pt_p = pio.tile([P, CC], F32, tag=t + "ptp")
            s.activation(pt_p[:], ce_p[:], Act.Exp, scale=-1.0)
            u_p = pio.tile([P, CC], F32, tag=t + "up")
            v.tensor_scalar(u_p[:], pt_p[:], -1.0, 1.0, op0=Alu.mult, op1=Alu.add)
            v.tensor_tensor(u_p[:], u_p[:], u_p[:], op=Alu.mult)
            foc_p = pio.tile([P, CC], F32, tag=t + "focp")
            v.scalar_tensor_tensor(foc_p[:], u_p[:], ALPHA, ce_p[:],
                                   op0=Alu.mult, op1=Alu.mult)
            sums4 = pio.tile([P, 4], F32, tag=t + "sums4")
            jnk = pio.tile([P, CC], F32, tag=t + "jnkcc")
            v.scalar_tensor_tensor(jnk[:], posf[:], 1.0, foc_p[:],
                                   op0=Alu.mult, op1=Alu.mult,
                                   accum_out=sums4[:, 0:1])

            # giou [P, CC]
            gx0 = cgt[:, :, 0]
            gy0 = cgt[:, :, 1]
            gx1 = cgt[:, :, 2]
            gy1 = cgt[:, :, 3]
            gw = pio.tile([P, CC], F32, tag=t + "gw")
            gh = pio.tile([P, CC], F32, tag=t + "gh")
            gA = pio.tile([P, CC], F32, tag=t + "gA")
            v.tensor_tensor(gw[:], gx1, gx0, op=Alu.subtract)
            v.tensor_tensor(gh[:], gy1, gy0, op=Alu.subtract)
            v.tensor_tensor(gA[:], gw[:], gh[:], op=Alu.mult)
            t0 = pio.tile([P, CC], F32, tag=t + "t0")
            t1 = pio.tile([P, CC], F32, tag=t + "t1")
            it_ = pio.tile([P, CC], F32, tag=t + "it")
            v.tensor_tensor(t0[:], cdx1[:], gx1, op=Alu.min)
            v.tensor_tensor(t1[:], cdx0[:], gx0, op=Alu.max)
            v.tensor_tensor(t0[:], t0[:], t1[:], op=Alu.subtract)
            v.tensor_scalar(t0[:], t0[:], 0.0, None, op0=Alu.max)
            v.tensor_tensor(it_[:], cdy1[:], gy1, op=Alu.min)
            v.tensor_tensor(t1[:], cdy0[:], gy0, op=Alu.max)
            v.tensor_tensor(it_[:], it_[:], t1[:], op=Alu.subtract)
            v.tensor_scalar(it_[:], it_[:], 0.0, None, op0=Alu.max)
            v.tensor_tensor(it_[:], it_[:], t0[:], op=Alu.mult)     # inter
            un = pio.tile([P, CC], F32, tag=t + "un")
            v.tensor_tensor(un[:], cAn[:], gA[:], op=Alu.add)
            v.tensor_tensor(un[:], un[:], it_[:], op=Alu.subtract)
            iou_d = pio.tile([P, CC], F32, tag=t + "ioud")
            v.tensor_scalar(iou_d[:], un[:], 1e-7, None, op0=Alu.max)
            v.reciprocal(iou_d[:], iou_d[:])
            v.tensor_tensor(iou_d[:], iou_d[:], it_[:], op=Alu.mult)
            # enclosure
            v.tensor_tensor(t0[:], cdx1[:], gx1, op=Alu.max)
            v.tensor_tensor(t1[:], cdx0[:], gx0, op=Alu.min)
            v.tensor_tensor(t0[:], t0[:], t1[:], op=Alu.subtract)
            v.tensor_tensor(it_[:], cdy1[:], gy1, op=Alu.max)
            v.tensor_tensor(t1[:], cdy0[:], gy0, op=Alu.min)
            v.tensor_tensor(it_[:], it_[:], t1[:], op=Alu.subtract)
            enc = pio.tile([P, CC], F32, tag=t + "enc")
            v.tensor_tensor(enc[:], t0[:], it_[:], op=Alu.mult)
            v.tensor_scalar(t0[:], enc[:], 1e-7, None, op0=Alu.max)
            v.reciprocal(t0[:], t0[:])
            v.tensor_tensor(enc[:], enc[:], un[:], op=Alu.subtract)
            v.tensor_tensor(enc[:], enc[:], t0[:], op=Alu.mult)
            cgi = pio.tile([P, CC], F32, tag=t + "cgi")
            v.tensor_tensor(cgi[:], iou_d[:], enc[:], op=Alu.subtract)
            v.tensor_scalar(cgi[:], cgi[:], -1.0, 1.0, op0=Alu.mult, op1=Alu.add)
            v.scalar_tensor_tensor(jnk[:], posf[:], 1.0, cgi[:],
                                   op0=Alu.mult, op1=Alu.mult,
                                   accum_out=sums4[:, 1:2])

            # ================= negatives =================
            vneg = pio.tile([P, F], BF16, tag=t + "vneg")
            tneg = pio.tile([P, F], F32, tag="tneg")
            v.tensor_scalar(tneg[:], negm[:], 2.0, -2.0, op0=Alu.mult, op1=Alu.add)
            v.tensor_tensor(vneg[:], tneg[:], sc[:], op=Alu.subtract)
            W8 = pio.tile([P, 8], BF16, tag=t + "W8")
            v.max(W8[:], vneg[:])
            W8f = pio.tile([P, 8], F32, tag=t + "W8f")
            v.tensor_copy(W8f[:], W8[:])
            wdr = pdr.tile([P, 8], BF16, tag=t + "wdr")
            nc.sync.dma_start(wdr[:], W8[:])
            wpool = pio.tile([P, P * 8], BF16, tag="wpool")
            nc.sync.dma_start(
                wpool[:],
                wdr[:].rearrange("p j -> (p j)")[None, :].broadcast_to([P, P * 8]))
            wr = pio.tile([P, 8], F32, tag=t + "wr")
            for jj in range(8):
                wscr = plp.tile([P, P * 8], F32, tag="wscr")
                v.tensor_scalar(wscr[:], wpool[:], W8f[:, jj:jj + 1], None,
                                op0=Alu.is_gt, op1=Alu.add, accum_out=wr[:, jj:jj + 1])
            km1 = pio.tile([P, 1], F32, tag=t + "km1")
            v.tensor_scalar(km1[:], kk[:], -1.0, None, op0=Alu.add)
            wsel = pio.tile([P, 8], F32, tag=t + "wsel")
            v.tensor_scalar(wsel[:], wr[:], km1[:], None, op0=Alu.is_le)
            v.tensor_scalar(wsel[:], wsel[:], -1.0, 1.0, op0=Alu.mult, op1=Alu.add)
            v.scalar_tensor_tensor(wsel[:], wsel[:], 1.0e30, W8f[:],
                                   op0=Alu.mult, op1=Alu.add)
            wmin = pio.tile([P, 1], F32, tag=t + "wmin")
            v.tensor_reduce(wmin[:], wsel[:], axis=mybir.AxisListType.X, op=Alu.min)
            # cross-partition min via transpose + reduce
            ps_w = pps.tile([1, P], F32, tag="psrow")
            nc.tensor.matmul(ps_w[:], wmin[:], idf[:])
            roww = pio.tile([1, P], F32, tag=t + "roww")
            s.activation(roww[:], ps_w[:], Act.Copy)
            tauv1 = pio.tile([1, 1], F32, tag=t + "tauv1")
            v.tensor_reduce(tauv1[:], roww[:], axis=mybir.AxisListType.X, op=Alu.min)
            ps_tw = pps.tile([P, P], F32, tag="psbc")
            bcast_row(ps_tw[:, 0:1], tauv1[:])
            tauv = pio.tile([P, 1], F32, tag=t + "tauv")
            s.activation(tauv[:], ps_tw[:, 0:1], Act.Copy)

            tap(t + "tauv", tauv[:1, :])
            tap(t + "kk", kk[:1, :])
            # dense neg focal
            ce_n = pio.tile([P, F], F32, tag="cen")
            v.tensor_tensor(ce_n[:], lse[:], cls0[:], op=Alu.subtract)
            pt_n = pio.tile([P, F], F32, tag="ptn")
            s.activation(pt_n[:], ce_n[:], Act.Exp, scale=-1.0)
            u_n = pio.tile([P, F], F32, tag="un2")
            v.tensor_scalar(u_n[:], pt_n[:], -1.0, 1.0, op0=Alu.mult, op1=Alu.add)
            u2_n = pio.tile([P, F], F32, tag="u2n")
            s.activation(u2_n[:], u_n[:], Act.Square)
            foc_n = pio.tile([P, F], F32, tag="focn")
            v.scalar_tensor_tensor(foc_n[:], u2_n[:], ALPHA, ce_n[:],
                                   op0=Alu.mult, op1=Alu.mult)
            selm = pio.tile([P, F], F32, tag="selm")
            v.tensor_scalar(selm[:], vneg[:], tauv[:], None, op0=Alu.is_ge)
            v.tensor_tensor(selm[:], selm[:], negm[:], op=Alu.mult)
            jnkF = pio.tile([P, F], F32, tag="jnkF")
            v.scalar_tensor_tensor(jnkF[:], selm[:], 1.0, foc_n[:],
                                   op0=Alu.mult, op1=Alu.mult,
                                   accum_out=sums4[:, 2:3])
            v.scalar_tensor_tensor(jnkF[:], negm[:], 1.0, foc_n[:],
                                   op0=Alu.mult, op1=Alu.mult,
                                   accum_out=sums4[:, 3:4])

            # ================= combine =================
            ps_s4 = pps.tile([1, P], F32, tag="psrow")
            colsum(ps_s4[:, 0:4], sums4[:])
            rows4 = pio.tile([1, 4], F32, tag=t + "rows4")
            v.tensor_copy(rows4[:], ps_s4[:, 0:4])
            pos_sum = rows4[:, 0:1]
            reg_sum = rows4[:, 1:2]
            sel_sum = rows4[:, 2:3]
            all_sum = rows4[:, 3:4]
            # scalars on partition 0 ([1,1] ops)
            np1 = pio.tile([1, 6], F32, tag=t + "np1")
            # np1: [num_pos, kk, nneg, tauv-unused, ...] rebuild from partition0 rows
            v.tensor_copy(np1[:, 0:1], num_pos[:1, 0:1])
            v.tensor_copy(np1[:, 1:2], kk[:1, 0:1])
            v.tensor_copy(np1[:, 2:3], nneg[:1, 0:1])
            branch = pio.tile([1, 1], F32, tag=t + "branch")
            v.tensor_scalar(branch[:], np1[:, 2:3], np1[:, 1:2], None, op0=Alu.is_gt)
            negsum = pio.tile([1, 1], F32, tag=t + "negsum")
            tt1 = pio.tile([1, 1], F32, tag=t + "tt1")
            v.tensor_tensor(tt1[:], sel_sum, all_sum, op=Alu.subtract)
            v.tensor_tensor(tt1[:], tt1[:], branch[:], op=Alu.mult)
            v.tensor_tensor(negsum[:], all_sum, tt1[:], op=Alu.add)
            negcnt = pio.tile([1, 1], F32, tag=t + "negcnt")
            v.tensor_tensor(tt1[:], np1[:, 1:2], np1[:, 2:3], op=Alu.subtract)
            v.tensor_tensor(tt1[:], tt1[:], branch[:], op=Alu.mult)
            v.tensor_tensor(negcnt[:], np1[:, 2:3], tt1[:], op=Alu.add)
            tots = pio.tile([1, 1], F32, tag=t + "tots")
            v.tensor_tensor(tots[:], np1[:, 0:1], negcnt[:], op=Alu.add)
            v.tensor_scalar(tots[:], tots[:], 1.0, None, op0=Alu.max)
            v.reciprocal(tots[:], tots[:])
            clsl = pio.tile([1, 1], F32, tag=t + "clsl")
            v.tensor_tensor(clsl[:], pos_sum, negsum[:], op=Alu.add)
            v.tensor_tensor(clsl[:], clsl[:], tots[:], op=Alu.mult)
            npc = pio.tile([1, 1], F32, tag=t + "npc")
            v.tensor_scalar(npc[:], np1[:, 0:1], 1.0, None, op0=Alu.max)
            v.reciprocal(npc[:], npc[:])
            regl = pio.tile([1, 1], F32, tag=t + "regl")
            v.tensor_tensor(regl[:], reg_sum, npc[:], op=Alu.mult)
            v.tensor_tensor(clsl[:], clsl[:], regl[:], op=Alu.add)
            v.tensor_tensor(acc_part[:], acc_part[:], clsl[:], op=Alu.add)

            tap(t + "rows4", rows4[:])
        nc.sync.dma_start(o_part[:], acc_part[:])


# ======================= host-side runner =======================
_CACHE = {}


def _split_multiwaits(bj):
    """This container's walrus supports one sync-wait per instruction; split
    Tile's multi-wait instructions into NoOp chains at BIR-JSON level."""
    import json
    m = json.loads(bj)
    for fn in m["functions"]:
        for b in fn["blocks"]:
            out = []
            for i in b.get("instructions", []):
                si = i.get("sync_info") or {}
                ow = si.get("on_wait") or []
                if len(ow) > 1:
                    for w_ix, w in enumerate(ow[:-1]):
                        out.append({"name": f"{i['name']}_w{w_ix}",
                                    "opcode": "NoOp", "engine": i["engine"],
                                    "ins": [], "outs": [],
                                    "sync_info": {"on_wait": [w],
                                                  "on_update": []}})
                    si["on_wait"] = [ow[-1]]
                out.append(i)
            b["instructions"] = out
    return json.dumps(m).encode()


def _install_bir_patch():
    import concourse.bass2jax as b2j
    if getattr(b2j, "_mw_patched", False):
        return
    orig = b2j.compile_bir_kernel

    def patched(bir_json, tmpdir, neff_name="file.neff"):
        return orig(_split_multiwaits(bir_json), tmpdir, neff_name=neff_name)

    b2j.compile_bir_kernel = patched
    b2j._mw_patched = True


def _get_nc():
    if "nc" in _CACHE:
        return _CACHE["nc"]
    import concourse.tile as tile
    nc = bass.Bass("TRN2", target_bir_lowering=False, debug=False)
    d_cls = nc.dram_tensor("d_cls", [IMGS, C, N], F32, kind="ExternalInput").ap()
    d_reg = nc.dram_tensor("d_reg", [IMGS, 4, N], F32, kind="ExternalInput").ap()
    d_anch = nc.dram_tensor("d_anch", [N * 4], F32, kind="ExternalInput").ap()
    d_tb = nc.dram_tensor("d_tb", [IMGS, M, 4], F32, kind="ExternalInput").ap()
    d_tl = nc.dram_tensor("d_tl", [IMGS, M], I32, kind="ExternalInput").ap()
    d_sc = nc.dram_tensor("d_sc", [IMGS, N], F32, kind="ExternalInput").ap()
    d_out = nc.dram_tensor("d_out", [1, 1], F32, kind="ExternalOutput").ap()
    with tile.TileContext(nc) as tc:
        build(nc, tc, [d_out], [d_cls, d_reg, d_anch, d_tb, d_tl, d_sc])
    _CACHE["nc"] = nc
    return nc


def _in_maps(cls_output, reg_output, anchors, target_boxes, target_labels,
             neg_scores, n_cores=8):
    B = cls_output.shape[0]
    assert B == n_cores * IMGS
    maps = []
    for cix in range(n_cores):
        i0 = cix * IMGS
        sl = slice(i0, i0 + IMGS)
        maps.append({
            "d_cls": np.ascontiguousarray(
                np.asarray(cls_output[sl], np.float32).reshape(IMGS, C, N)),
            "d_reg": np.ascontiguousarray(
                np.asarray(reg_output[sl], np.float32).reshape(IMGS, 4, N)),
            "d_anch": np.ascontiguousarray(
                np.asarray(anchors, np.float32).reshape(N * 4)),
            "d_tb": np.ascontiguousarray(
                np.asarray(target_boxes[sl], np.float32)),
            "d_tl": np.ascontiguousarray(
                np.asarray(target_labels[sl]).astype(np.int32)),
            "d_sc": np.ascontiguousarray(
                np.asarray(neg_scores[sl], np.float32)),
        })
    return maps


def kernel(cls_output, reg_output, anchors, target_boxes, target_labels,
           neg_scores):
    from concourse.bass_utils import run_bass_kernel_spmd
    _install_bir_patch()
    nc = _get_nc()
    maps = _in_maps(cls_output, reg_output, anchors, target_boxes,
                    target_labels, neg_scores)
    res = run_bass_kernel_spmd(nc, maps, core_ids=list(range(8)))
    B = cls_output.shape[0]
    total = sum(float(r["d_out"][0, 0]) for r in res.results) / B
    return np.array(total, dtype=np.float32)
